# revision 1
# baseline (speedup 1.0000x reference)
"""Multi-head attention (B=2, S=4096, E=768, H=12, D=64) on 8 TRN2 NeuronCores.

Sharding: data parallel over batch (2) x tensor parallel over head groups (4):
core c handles batch c//4, heads 3*(c%4) .. 3*(c%4)+2.

Per-core device kernel (fp16 matmul inputs, fp32 accumulation):
  phase 1: Q^T,K^T [192,4096] and V [4096,192] projections from x^T
  phase 2: per (head, q-block of 512): S^T tiles = K^T_h.T @ Q^T_h (k on
    partitions), exp via ScalarE (scale=1/8 folded in), attn@V with a ones
    column appended to V so the softmax denominator falls out of the same
    matmul, normalize via reciprocal + K=1 broadcast matmul, then the
    row-parallel output projection producing a partial y^T.
Host: y[b] = sum of the 4 partial y^T.T per batch + b_proj.
"""
import numpy as np

EMBED = 768
SEQ = 4096
NHEAD_CORE = 3          # heads per core
DHEAD = 64
DSL = NHEAD_CORE * DHEAD  # 192: per-core head-dim slice
QB = 512                # q-block (free dim per PSUM bank)
NQB = SEQ // QB         # 8
NKT = SEQ // 128        # 32 k-tiles
NEC = EMBED // 128      # 6 e-chunks
NFT = EMBED // 128      # 6 f-tiles
SCALE = DHEAD ** -0.5

_CACHED = {}


def _build():
    import concourse.bacc as bacc
    import concourse.tile as tile
    from concourse import mybir

    F32 = mybir.dt.float32
    F16 = mybir.dt.float16
    EXP = mybir.ActivationFunctionType.Exp

    nc = bacc.Bacc("TRN2")
    xT_d = nc.dram_tensor("xT", [EMBED, SEQ], F16, kind="ExternalInput")
    wq_d = nc.dram_tensor("wq", [EMBED, DSL], F16, kind="ExternalInput")
    wk_d = nc.dram_tensor("wk", [EMBED, DSL], F16, kind="ExternalInput")
    wv_d = nc.dram_tensor("wv", [EMBED, DSL], F16, kind="ExternalInput")
    wp_d = nc.dram_tensor("wp", [DSL, EMBED], F16, kind="ExternalInput")
    yT_d = nc.dram_tensor("yT", [EMBED, SEQ], mybir.dt.float32, kind="ExternalOutput")

    with tile.TileContext(nc) as tc:
        with (
            tc.tile_pool(name="persist", bufs=1) as persist,
            tc.tile_pool(name="slab", bufs=6) as slabp,
            tc.tile_pool(name="es", bufs=8) as esp,
            tc.tile_pool(name="att", bufs=2) as attp,
            tc.tile_pool(name="rsbp", bufs=2) as rsbp,
            tc.tile_pool(name="bcp", bufs=2) as bcp,
            tc.tile_pool(name="stage", bufs=3) as stagep,
            tc.tile_pool(name="psA", bufs=2, space="PSUM") as psA,
            tc.tile_pool(name="psB", bufs=2, space="PSUM") as psB,
            tc.tile_pool(name="psC", bufs=2, space="PSUM") as psC,
        ):
            # ---- persistent SBUF ----
            wq_sb = persist.tile([128, NEC, DSL], F16, name="wq_sb")
            wk_sb = persist.tile([128, NEC, DSL], F16, name="wk_sb")
            wv_sb = persist.tile([128, NEC, DSL], F16, name="wv_sb")
            wp_a = persist.tile([128, EMBED], F16, name="wp_a")
            wp_b = persist.tile([128, EMBED], F16, name="wp_b")
            # Q^T/K^T packed pairs: t0 = [Qh0|Qh1], t1 = [Kh0|Kh1],
            # t2 = [Qh2|-], t3 = [Kh2|-]
            qt01 = persist.tile([128, SEQ], F16, name="qt01")
            kt01 = persist.tile([128, SEQ], F16, name="kt01")
            qt2 = persist.tile([128, SEQ], F16, name="qt2")
            kt2 = persist.tile([128, SEQ], F16, name="kt2")
            # V natural layout + ones column: [p, head, kchunk, 66]
            v_sb = persist.tile([128, NHEAD_CORE, NKT, 66], F16, name="v_sb")
            ones_sb = persist.tile([128, 64], F16, name="ones_sb")

            wqr = wq_d.rearrange("(c p) d -> p c d", p=128)
            wkr = wk_d.rearrange("(c p) d -> p c d", p=128)
            wvr = wv_d.rearrange("(c p) d -> p c d", p=128)
            nc.sync.dma_start(out=wq_sb[:], in_=wqr)
            nc.sync.dma_start(out=wk_sb[:], in_=wkr)
            nc.sync.dma_start(out=wv_sb[:], in_=wvr)
            nc.sync.dma_start(out=wp_a[:], in_=wp_d[0:128, :])
            nc.sync.dma_start(out=wp_b[0:64, :], in_=wp_d[128:192, :])
            nc.vector.memset(ones_sb[:], 1.0)
            nc.vector.memset(v_sb[:, :, :, 64:66], 1.0)

            # ---- phase 1: Q^T, K^T, V projections ----
            # QK: lhsT = w slice [e-chunk, m-cols], rhs = xT slab [e-chunk, s-block]
            # V:  lhsT = xT slab s-chunk [e, 128 s], rhs = wv slice [e, 192]
            for sb in range(NQB):  # 8 s-blocks of 512
                qk_ps = [psA.tile([128, 1024], F32, name="qkps", tag="psA")
                         for _ in range(2)]
                v_ps = [
                    psB.tile([128, QB], F32, name="vps0", tag="psB"),
                    psB.tile([128, QB], F32, name="vps1", tag="psB"),
                    psC.tile([128, QB], F32, name="vps2", tag="psC"),
                    psC.tile([128, QB], F32, name="vps3", tag="psC"),
                ]
                for e in range(NEC):
                    slab = slabp.tile([128, QB], F16, name="slab", tag="slab")
                    nc.sync.dma_start(
                        out=slab[:],
                        in_=xT_d[128 * e:128 * (e + 1), QB * sb:QB * (sb + 1)],
                    )
                    st = (e == 0)
                    sp = (e == NEC - 1)
                    # Q heads 0,1 -> qk_ps[0][:, 0:512]
                    nc.tensor.matmul(qk_ps[0][:, 0:QB], wq_sb[:, e, 0:128],
                                     slab[:], start=st, stop=sp)
                    # K heads 0,1 -> qk_ps[0][:, 512:1024]
                    nc.tensor.matmul(qk_ps[0][:, QB:2 * QB], wk_sb[:, e, 0:128],
                                     slab[:], start=st, stop=sp)
                    # Q head 2 -> qk_ps[1][:, 0:512] (M=64)
                    nc.tensor.matmul(qk_ps[1][0:64, 0:QB], wq_sb[:, e, 128:192],
                                     slab[:], start=st, stop=sp)
                    # K head 2 -> qk_ps[1][:, 512:1024] (M=64)
                    nc.tensor.matmul(qk_ps[1][0:64, QB:2 * QB], wk_sb[:, e, 128:192],
                                     slab[:], start=st, stop=sp)
                    # V: 4 s-chunks of 128
                    for c in range(4):
                        nc.tensor.matmul(
                            v_ps[c][:, 0:DSL],
                            slab[:, 128 * c:128 * (c + 1)],
                            wv_sb[:, e, :],
                            start=st, stop=sp)
                # evacuate
                cols = slice(QB * sb, QB * (sb + 1))
                nc.vector.tensor_copy(qt01[:, cols], qk_ps[0][:, 0:QB])
                nc.vector.tensor_copy(kt01[:, cols], qk_ps[0][:, QB:2 * QB])
                nc.vector.tensor_copy(qt2[0:64, cols], qk_ps[1][0:64, 0:QB])
                nc.vector.tensor_copy(kt2[0:64, cols], qk_ps[1][0:64, QB:2 * QB])
                for c in range(4):
                    kc = 4 * sb + c  # global k-chunk
                    nc.vector.tensor_copy(
                        v_sb[:, :, kc, 0:64],
                        v_ps[c][:, 0:DSL].rearrange("p (h d) -> p h d", h=NHEAD_CORE),
                    )

            # per-head (lhsT_K, rhs_Q, base) slices
            heads = [
                (kt01, qt01, 0),
                (kt01, qt01, 64),
                (kt2, qt2, 0),
            ]

            # ---- phase 2: attention + projection ----
            for qb in range(NQB):
                qcols = slice(QB * qb, QB * (qb + 1))
                att_ab = [
                    attp.tile([128, QB], F16, name="attA", tag="attA"),
                    attp.tile([128, QB], F16, name="attB", tag="attB"),
                ]
                for h in range(NHEAD_CORE):
                    kt_t, qt_t, b0 = heads[h]
                    ps_att = psB.tile([128, QB], F32, name="ps_att", tag="psB")
                    for j in range(NKT // 2):  # 16 pairs of k-tiles
                        ps_s = psA.tile([128, 1024], F32, name="ps_s", tag="psA")
                        for u in range(2):
                            kt_i = 2 * j + u
                            nc.tensor.matmul(
                                ps_s[:, QB * u:QB * (u + 1)],
                                kt_t[b0:b0 + 64, 128 * kt_i:128 * (kt_i + 1)],
                                qt_t[b0:b0 + 64, qcols],
                                start=True, stop=True)
                        es = esp.tile([128, 1024], F16, name="es", tag="es")
                        nc.scalar.activation(out=es[:], in_=ps_s[:],
                                             func=EXP, scale=SCALE)
                        for u in range(2):
                            kt_i = 2 * j + u
                            nc.tensor.matmul(
                                ps_att[0:65, :],
                                v_sb[:, h, kt_i, 0:65],
                                es[:, QB * u:QB * (u + 1)],
                                start=(kt_i == 0), stop=(kt_i == NKT - 1),
                                skip_group_check=True)
                    # normalize: recip of denom row 64, broadcast via K=1 matmul
                    rsb = rsbp.tile([128, QB], F16, name="rsb", tag="rsb")
                    with nc.allow_low_precision(reason="fp16 recip feeds bcast mm"):
                        nc.vector.reciprocal(out=rsb[64:65, :], in_=ps_att[64:65, :])
                    ps_bc = psC.tile([128, QB], F32, name="ps_bc", tag="psC")
                    nc.tensor.matmul(ps_bc[0:64, :], ones_sb[64:65, 0:64],
                                     rsb[64:65, :], start=True, stop=True)
                    bc_sb = bcp.tile([128, QB], F32, name="bc_sb", tag="bc")
                    nc.vector.tensor_copy(bc_sb[0:64, :], ps_bc[0:64, :])
                    # write normalized attn output into proj-ready slots
                    if h == 0:
                        dst = att_ab[0][0:64, :]
                    elif h == 1:
                        dst = att_ab[0][64:128, :]
                    else:
                        dst = att_ab[1][0:64, :]
                    nc.vector.tensor_mul(dst, ps_att[0:64, :], bc_sb[0:64, :])

                # output projection: yT[f,:] += wp.T @ attnout
                for f in range(NFT):
                    ps_o = psC.tile([128, QB], F32, name="ps_o", tag="psC")
                    nc.tensor.matmul(ps_o[:], wp_a[:, 128 * f:128 * (f + 1)],
                                     att_ab[0][:], start=True, stop=False)
                    nc.tensor.matmul(ps_o[:], wp_b[0:64, 128 * f:128 * (f + 1)],
                                     att_ab[1][0:64, :], start=False, stop=True)
                    stg = stagep.tile([128, QB], mybir.dt.float32,
                                      name="stg", tag="stg")
                    nc.vector.tensor_copy(stg[:], ps_o[:])
                    nc.sync.dma_start(
                        out=yT_d[128 * f:128 * (f + 1), qcols], in_=stg[:])

    nc.compile()
    return nc


def _get_nc():
    if "nc" not in _CACHED:
        _CACHED["nc"] = _build()
    return _CACHED["nc"]


def _make_in_maps(x, W_qkv, W_proj):
    f16 = np.float16
    in_maps = []
    for c in range(8):
        b = c // 4
        g = c % 4
        sl = slice(DSL * g, DSL * (g + 1))
        xT = np.ascontiguousarray(x[b].T).astype(f16)
        wq = np.ascontiguousarray(W_qkv[0:EMBED][sl, :].T).astype(f16)
        wk = np.ascontiguousarray(W_qkv[EMBED:2 * EMBED][sl, :].T).astype(f16)
        wv = np.ascontiguousarray(W_qkv[2 * EMBED:3 * EMBED][sl, :].T).astype(f16)
        wp = np.ascontiguousarray(W_proj[:, sl].T).astype(f16)
        in_maps.append({"xT": xT, "wq": wq, "wk": wk, "wv": wv, "wp": wp})
    return in_maps


def kernel(x, W_qkv, W_proj, b_proj):
    from concourse.bass_utils import run_bass_kernel_spmd

    x = np.asarray(x, dtype=np.float32)
    W_qkv = np.asarray(W_qkv, dtype=np.float32)
    W_proj = np.asarray(W_proj, dtype=np.float32)
    b_proj = np.asarray(b_proj, dtype=np.float32)

    nc = _get_nc()
    in_maps = _make_in_maps(x, W_qkv, W_proj)
    res = run_bass_kernel_spmd(nc, in_maps, core_ids=list(range(8)))

    y = np.zeros((2, SEQ, EMBED), dtype=np.float32)
    for c in range(8):
        y[c // 4] += res.results[c]["yT"].T
    y += b_proj
    return y


# revision 24
# speedup vs baseline: 1.8569x; 1.8569x over previous
"""Multi-head attention (B=2, S=4096, E=768, H=12, D=64) on 8 TRN2 NeuronCores.

Sharding: data parallel over batch (2) x tensor parallel over head groups (4):
core c handles batch c//4, heads 3*(c%4) .. 3*(c%4)+2.

Per-core device kernel (fp16 matmul inputs, fp32 accumulation):
  phase 1: Q^T,K^T [192,4096] and V [4096,192] projections from x^T.
    Heads 0,1 of the group are packed at partitions 0-63/64-127 of shared
    Q^T/K^T tiles; head 2's Q and K share one M=128 stationary (host passes
    the combined weight block).
  phase 2: per (q-block of 512, head) block: 3 k-tiles of scores per exp op
    (3 PSUM banks; ScalarE with the 1/8 scale folded in); even/odd k-tiles
    run at partition bases 0/64 (via swapped Q/K duplicates) so adjacent
    K=64 score matmuls occupy different PE row groups and overlap in HW.
    attn@V carries a ones column so the softmax denominator falls out of the
    same matmul; the accumulator spills to SBUF at block end (frees its PSUM
    bank), and normalize (reciprocal + K=1 broadcast matmul) plus the
    row-parallel output projection ride in the next block's slack.
Host: y[b] = sum of the 4 partial y^T.T per batch + b_proj.
"""
import numpy as np

EMBED = 768
SEQ = 4096
NHEAD_CORE = 3          # heads per core
DHEAD = 64
DSL = NHEAD_CORE * DHEAD  # 192: per-core head-dim slice
QB = 512                # q-block (free dim per PSUM bank)
NQB = SEQ // QB         # 8
NKT = SEQ // 128        # 32 k-tiles
NEC = EMBED // 128      # 6 e-chunks
NFT = EMBED // 128      # 6 f-tiles
SCALE = DHEAD ** -0.5

_CACHED = {}


def _build():
    import concourse.bacc as bacc
    import concourse.tile as tile
    from concourse import mybir

    F32 = mybir.dt.float32
    F16 = mybir.dt.float16
    EXP = mybir.ActivationFunctionType.Exp

    nc = bacc.Bacc("TRN2")
    xT_d = nc.dram_tensor("xT", [EMBED, SEQ], F16, kind="ExternalInput")
    wq_d = nc.dram_tensor("wq", [EMBED, 128], F16, kind="ExternalInput")
    wk_d = nc.dram_tensor("wk", [EMBED, 128], F16, kind="ExternalInput")
    wqk2_d = nc.dram_tensor("wqk2", [EMBED, 128], F16, kind="ExternalInput")
    wv_d = nc.dram_tensor("wv", [EMBED, DSL], F16, kind="ExternalInput")
    wp_d = nc.dram_tensor("wp", [DSL, EMBED], F16, kind="ExternalInput")
    yT_d = nc.dram_tensor("yT", [EMBED, SEQ], F32, kind="ExternalOutput")

    with tile.TileContext(nc) as tc:
        with (
            tc.tile_pool(name="persist", bufs=1) as persist,
            tc.tile_pool(name="slab", bufs=12) as slabp,
            tc.tile_pool(name="es", bufs=12) as esp,
            tc.tile_pool(name="att", bufs=2) as attp,
            tc.tile_pool(name="rsbp", bufs=4) as rsbp,
            tc.tile_pool(name="bcp", bufs=4) as bcp,
            tc.tile_pool(name="stage", bufs=3) as stagep,
            tc.tile_pool(name="spill", bufs=2) as spillp,
            tc.tile_pool(name="psA", bufs=2, space="PSUM") as psA,
            tc.tile_pool(name="psB", bufs=1, space="PSUM") as psB,
            tc.tile_pool(name="psC", bufs=1, space="PSUM") as psC,
        ):
            # ---- persistent SBUF ----
            wq_sb = persist.tile([128, NEC, 128], F16, name="wq_sb")
            wk_sb = persist.tile([128, NEC, 128], F16, name="wk_sb")
            wqk2_sb = persist.tile([128, NEC, 128], F16, name="wqk2_sb")
            wv_sb = persist.tile([128, NEC, DSL], F16, name="wv_sb")
            wp_a = persist.tile([128, EMBED], F16, name="wp_a")
            wp_b = persist.tile([128, EMBED], F16, name="wp_b")
            # Q^T/K^T: heads 0,1 at partition halves; head 2 at base 0
            qt01 = persist.tile([128, SEQ], F16, name="qt01")
            kt01 = persist.tile([128, SEQ], F16, name="kt01")
            qt2 = persist.tile([128, SEQ], F16, name="qt2")
            kt2 = persist.tile([128, SEQ], F16, name="kt2")
            # swapped duplicates: [h1 | h0] so every head has Q/K at both
            # partition halves (even k-tiles run at base 0, odd at base 64 ->
            # adjacent matmuls occupy different PE row groups and overlap)
            qtdup = persist.tile([128, SEQ], F16, name="qtdup")
            ktdup = persist.tile([128, SEQ], F16, name="ktdup")
            # V natural layout + ones column: [p, head, kchunk, 66]
            v_sb = persist.tile([128, NHEAD_CORE, NKT, 66], F16, name="v_sb")
            ones_sb = persist.tile([128, 64], F16, name="ones_sb")

            nc.sync.dma_start(out=wq_sb[:], in_=wq_d.rearrange("(c p) d -> p c d", p=128))
            nc.sync.dma_start(out=wk_sb[:], in_=wk_d.rearrange("(c p) d -> p c d", p=128))
            nc.vector.memset(ones_sb[:], 1.0)
            nc.vector.memset(v_sb[:, :, :, 64:66], 1.0)
            # dummy activation: loads the exp table set while ACT is idle
            warm = persist.tile([128, 1], F16, name="warm_sb")
            nc.vector.memset(warm[:], 0.0)
            nc.scalar.activation(out=warm[:], in_=warm[:], func=EXP, scale=1.0)

            # ---- phase 1, pass A: Q^T/K^T for heads 0,1 (gets ACT going fast)
            for sb in range(NQB):  # 8 s-blocks of 512
                qk_ps = psA.tile([128, 1024], F32, name="qkps", tag="psA")
                for e in range(NEC):
                    slab = slabp.tile([128, QB], F16, name="slab", tag="slab")
                    nc.sync.dma_start(
                        out=slab[:],
                        in_=xT_d[128 * e:128 * (e + 1), QB * sb:QB * (sb + 1)],
                    )
                    st = (e == 0)
                    sp = (e == NEC - 1)
                    nc.tensor.matmul(qk_ps[:, 0:QB], wq_sb[:, e, :],
                                     slab[:], start=st, stop=sp)
                    nc.tensor.matmul(qk_ps[:, QB:2 * QB], wk_sb[:, e, :],
                                     slab[:], start=st, stop=sp)
                cols = slice(QB * sb, QB * (sb + 1))
                nc.vector.tensor_copy(qt01[:, cols], qk_ps[:, 0:QB])
                nc.vector.tensor_copy(kt01[:, cols], qk_ps[:, QB:2 * QB])
                nc.vector.tensor_copy(qtdup[64:128, cols], qk_ps[0:64, 0:QB])
                nc.vector.tensor_copy(ktdup[64:128, cols], qk_ps[0:64, QB:2 * QB])

            nc.sync.dma_start(out=wqk2_sb[:], in_=wqk2_d.rearrange("(c p) d -> p c d", p=128))
            nc.sync.dma_start(out=wv_sb[:], in_=wv_d.rearrange("(c p) d -> p c d", p=128))
            nc.sync.dma_start(out=wp_a[:], in_=wp_d[0:128, :])
            nc.sync.dma_start(out=wp_b[0:64, :], in_=wp_d[128:192, :])

            # phase 1, pass B (emitted in bursts inside q-block 0's loop):
            # {Q2|K2} projection + V projection for one s-block
            def pass_b_burst(sb):
                cols = slice(QB * sb, QB * (sb + 1))
                slabs = []
                for e in range(NEC):
                    slab = slabp.tile([128, QB], F16, name="slabB", tag="slab")
                    nc.sync.dma_start(
                        out=slab[:],
                        in_=xT_d[128 * e:128 * (e + 1), cols],
                    )
                    slabs.append(slab)
                qk2_ps = psA.tile([128, 1536], F32, name="qk2ps", tag="psA")
                for e in range(NEC):
                    nc.tensor.matmul(qk2_ps[:, 0:QB], wqk2_sb[:, e, :],
                                     slabs[e][:], start=(e == 0),
                                     stop=(e == NEC - 1))
                nc.vector.tensor_copy(qt2[0:64, cols], qk2_ps[0:64, 0:QB])
                nc.vector.tensor_copy(kt2[0:64, cols], qk2_ps[64:128, 0:QB])
                nc.vector.tensor_copy(qt2[64:128, cols], qk2_ps[0:64, 0:QB])
                nc.vector.tensor_copy(kt2[64:128, cols], qk2_ps[64:128, 0:QB])
                for c in range(4):  # V s-chunks, one PSUM bank at a time
                    v_ps = psC.tile([128, QB], F32, name="vps", tag="psC")
                    for e in range(NEC):
                        nc.tensor.matmul(
                            v_ps[:, 0:DSL],
                            slabs[e][:, 128 * c:128 * (c + 1)],
                            wv_sb[:, e, :],
                            start=(e == 0), stop=(e == NEC - 1))
                    nc.vector.tensor_copy(
                        v_sb[:, :, 4 * sb + c, 0:64],
                        v_ps[:, 0:DSL].rearrange("p (h d) -> p h d", h=NHEAD_CORE),
                    )

            # ---- phase 2: attention + projection ----
            # per-(qb, head) blocks; 3 k-tiles per exp op (3 PSUM banks);
            # attn accumulator spilled to SBUF at block end so psB needs one
            # bank; normalize + projection ride in the next block's slack.
            def normalize_sb(spill, dst):
                """dst = spill[0:64] / spill[64] via recip + K=1 bcast mm."""
                rsb = rsbp.tile([128, QB], F16, name="rsb", tag="rsb")
                with nc.allow_low_precision(reason="fp16 recip feeds bcast mm"):
                    nc.vector.reciprocal(out=rsb[64:65, :], in_=spill[64:65, :])
                ps_bc = psC.tile([128, QB], F32, name="ps_bc", tag="psC")
                nc.tensor.matmul(ps_bc[0:64, :], ones_sb[64:65, 0:64],
                                 rsb[64:65, :], start=True, stop=True)
                bc_sb = bcp.tile([128, QB], F32, name="bc_sb", tag="bc")
                nc.vector.tensor_copy(bc_sb[0:64, :], ps_bc[0:64, :])
                nc.vector.tensor_mul(dst, spill[0:64, :], bc_sb[0:64, :])

            def emit_proj(qb, attA, attB, f):
                qcols = slice(QB * qb, QB * (qb + 1))
                ps_o = psC.tile([128, QB], F32, name="ps_o", tag="psC")
                nc.tensor.matmul(ps_o[:], wp_a[:, 128 * f:128 * (f + 1)],
                                 attA[:], start=True, stop=False)
                nc.tensor.matmul(ps_o[:], wp_b[0:64, 128 * f:128 * (f + 1)],
                                 attB[0:64, :], start=False, stop=True)
                stg = stagep.tile([128, QB], F32, name="stg", tag="stg")
                nc.vector.tensor_copy(stg[:], ps_o[:])
                nc.sync.dma_start(
                    out=yT_d[128 * f:128 * (f + 1), qcols], in_=stg[:])

            # per-head (K even-base-0, Q even, K odd-base-64, Q odd) sources
            def head_srcs(h, kt):
                if kt % 2 == 0:
                    b0 = 0
                    kt_t, qt_t = [(kt01, qt01), (ktdup, qtdup), (kt2, qt2)][h]
                else:
                    b0 = 64
                    kt_t, qt_t = [(ktdup, qtdup), (kt01, qt01), (kt2, qt2)][h]
                return kt_t, qt_t, b0

            GROUPS = [list(range(i, min(i + 3, NKT))) for i in range(0, NKT, 3)]
            att_tiles = {}
            pend_norm = None   # (spill_tile, dst_ap)
            pend_proj = None   # (qb, attA, attB)
            next_burst = 0

            for qb in range(NQB):
                qcols = slice(QB * qb, QB * (qb + 1))
                attA = attp.tile([128, QB], F16, name="attA", tag="attA")
                attB = attp.tile([128, QB], F16, name="attB", tag="attB")
                att_tiles[qb] = (attA, attB)
                for h in range(NHEAD_CORE):
                    ps_att = psB.tile([128, QB], F32, name="ps_att", tag="psB")
                    for gi, group in enumerate(GROUPS):
                        if qb == 0 and h == 0 and next_burst < NQB:
                            # one burst per group: burst sb covers V k-chunks
                            # up to 4*sb+3 >= group[-1]=3*sb+2, always ahead
                            pass_b_burst(next_burst)
                            next_burst += 1
                        gw = QB * len(group)
                        ps_s = psA.tile([128, 1536], F32, name="ps_s", tag="psA")
                        for i, kt in enumerate(group):
                            kt_t, qt_t, b0 = head_srcs(h, kt)
                            kk = slice(128 * kt, 128 * (kt + 1))
                            nc.tensor.matmul(
                                ps_s[:, QB * i:QB * (i + 1)],
                                kt_t[b0:b0 + 64, kk], qt_t[b0:b0 + 64, qcols],
                                start=True, stop=True)
                        es = esp.tile([128, 1536], F16, name="es", tag="es")
                        nc.scalar.activation(out=es[:, 0:gw], in_=ps_s[:, 0:gw],
                                             func=EXP, scale=SCALE)
                        for i, kt in enumerate(group):
                            nc.tensor.matmul(
                                ps_att[0:65, :], v_sb[:, h, kt, 0:65],
                                es[:, QB * i:QB * (i + 1)],
                                start=(kt == 0), stop=(kt == NKT - 1),
                                skip_group_check=True)
                        if qb == 0 and h == 0 and 2 <= gi < 6:
                            # h1-side Q/K duplicates, first needed next block
                            srcs = [(qtdup, qt01), (ktdup, kt01)]
                            dt_, st_ = srcs[(gi - 2) % 2]
                            half = slice(0, SEQ // 2) if gi < 4 else slice(SEQ // 2, SEQ)
                            nc.vector.tensor_copy(dt_[0:64, half],
                                                  st_[64:128, half])
                        if gi == 1 and pend_norm is not None:
                            normalize_sb(*pend_norm)
                            pend_norm = None
                        if pend_proj is not None and 3 <= gi < 3 + NFT:
                            emit_proj(pend_proj[0], pend_proj[1], pend_proj[2],
                                      gi - 3)
                            if gi == 3 + NFT - 1:
                                pend_proj = None
                    # spill accumulator to SBUF; frees the psB bank quickly
                    spill = spillp.tile([128, QB], F32, name="spill", tag="spill")
                    nc.vector.tensor_copy(spill[0:65, :], ps_att[0:65, :])
                    if h == 0:
                        dst = attA[0:64, :]
                    elif h == 1:
                        dst = attA[64:128, :]
                    else:
                        dst = attB[0:64, :]
                    pend_norm = (spill, dst)
                    if h == 2:
                        pend_proj = (qb, attA, attB)

            normalize_sb(*pend_norm)
            for f in range(NFT):
                emit_proj(pend_proj[0], pend_proj[1], pend_proj[2], f)

    nc.compile()
    return nc


def _get_nc():
    if "nc" not in _CACHED:
        _CACHED["nc"] = _build()
    return _CACHED["nc"]


def _make_in_maps(x, W_qkv, W_proj):
    f16 = np.float16
    in_maps = []
    for c in range(8):
        b = c // 4
        g = c % 4
        sl = slice(DSL * g, DSL * (g + 1))
        xT = np.ascontiguousarray(x[b].T).astype(f16)
        wqT = np.ascontiguousarray(W_qkv[0:EMBED][sl, :].T)         # [768,192]
        wkT = np.ascontiguousarray(W_qkv[EMBED:2 * EMBED][sl, :].T)
        wvT = np.ascontiguousarray(W_qkv[2 * EMBED:3 * EMBED][sl, :].T)
        wp = np.ascontiguousarray(W_proj[:, sl].T)                  # [192,768]
        wqk2 = np.concatenate([wqT[:, 128:192], wkT[:, 128:192]], axis=1)
        in_maps.append({
            "xT": xT,
            "wq": wqT[:, 0:128].astype(f16),
            "wk": wkT[:, 0:128].astype(f16),
            "wqk2": np.ascontiguousarray(wqk2).astype(f16),
            "wv": wvT.astype(f16),
            "wp": wp.astype(f16),
        })
    return in_maps


def kernel(x, W_qkv, W_proj, b_proj):
    from concourse.bass_utils import run_bass_kernel_spmd

    x = np.asarray(x, dtype=np.float32)
    W_qkv = np.asarray(W_qkv, dtype=np.float32)
    W_proj = np.asarray(W_proj, dtype=np.float32)
    b_proj = np.asarray(b_proj, dtype=np.float32)

    nc = _get_nc()
    in_maps = _make_in_maps(x, W_qkv, W_proj)
    res = run_bass_kernel_spmd(nc, in_maps, core_ids=list(range(8)))

    y = np.zeros((2, SEQ, EMBED), dtype=np.float32)
    for c in range(8):
        y[c // 4] += res.results[c]["yT"].T
    y += b_proj
    return y


# revision 27
# speedup vs baseline: 1.8688x; 1.0064x over previous
"""Multi-head attention (B=2, S=4096, E=768, H=12, D=64) on 8 TRN2 NeuronCores.

Sharding: data parallel over batch (2) x tensor parallel over head groups (4):
core c handles batch c//4, heads 3*(c%4) .. 3*(c%4)+2.

Per-core device kernel (fp16 matmul inputs, fp32 accumulation):
  phase 1: Q^T,K^T [192,4096] and V [4096,192] projections from x^T.
    Heads 0,1 of the group are packed at partitions 0-63/64-127 of shared
    Q^T/K^T tiles; head 2's Q and K share one M=128 stationary (host passes
    the combined weight block).
  phase 2: per (q-block of 512, head) block: 3 k-tiles of scores per exp op
    (3 PSUM banks; ScalarE with the 1/8 scale folded in); even/odd k-tiles
    run at partition bases 0/64 (via swapped Q/K duplicates) so adjacent
    K=64 score matmuls occupy different PE row groups and overlap in HW.
    attn@V carries a ones column so the softmax denominator falls out of the
    same matmul; the accumulator spills to SBUF at block end (frees its PSUM
    bank), and normalize (reciprocal + K=1 broadcast matmul) plus the
    row-parallel output projection ride in the next block's slack.
Host: y[b] = sum of the 4 partial y^T.T per batch + b_proj.
"""
import numpy as np

EMBED = 768
SEQ = 4096
NHEAD_CORE = 3          # heads per core
DHEAD = 64
DSL = NHEAD_CORE * DHEAD  # 192: per-core head-dim slice
QB = 512                # q-block (free dim per PSUM bank)
NQB = SEQ // QB         # 8
NKT = SEQ // 128        # 32 k-tiles
NEC = EMBED // 128      # 6 e-chunks
NFT = EMBED // 128      # 6 f-tiles
SCALE = DHEAD ** -0.5

_CACHED = {}


def _build():
    import concourse.bacc as bacc
    import concourse.tile as tile
    from concourse import mybir

    F32 = mybir.dt.float32
    F16 = mybir.dt.float16
    EXP = mybir.ActivationFunctionType.Exp

    nc = bacc.Bacc("TRN2")
    xT_d = nc.dram_tensor("xT", [EMBED, SEQ], F16, kind="ExternalInput")
    wq_d = nc.dram_tensor("wq", [EMBED, 128], F16, kind="ExternalInput")
    wk_d = nc.dram_tensor("wk", [EMBED, 128], F16, kind="ExternalInput")
    wqk2_d = nc.dram_tensor("wqk2", [EMBED, 128], F16, kind="ExternalInput")
    wv_d = nc.dram_tensor("wv", [EMBED, DSL], F16, kind="ExternalInput")
    wp_d = nc.dram_tensor("wp", [DSL, EMBED], F16, kind="ExternalInput")
    yT_d = nc.dram_tensor("yT", [EMBED, SEQ], F32, kind="ExternalOutput")

    with tile.TileContext(nc) as tc:
        with (
            tc.tile_pool(name="persist", bufs=1) as persist,
            tc.tile_pool(name="slab", bufs=12) as slabp,
            tc.tile_pool(name="es", bufs=12) as esp,
            tc.tile_pool(name="att", bufs=2) as attp,
            tc.tile_pool(name="rsbp", bufs=4) as rsbp,
            tc.tile_pool(name="bcp", bufs=4) as bcp,
            tc.tile_pool(name="stage", bufs=3) as stagep,
            tc.tile_pool(name="spill", bufs=2) as spillp,
            tc.tile_pool(name="psA", bufs=2, space="PSUM") as psA,
            tc.tile_pool(name="psB", bufs=1, space="PSUM") as psB,
            tc.tile_pool(name="psC", bufs=1, space="PSUM") as psC,
        ):
            # ---- persistent SBUF ----
            wq_sb = persist.tile([128, NEC, 128], F16, name="wq_sb")
            wk_sb = persist.tile([128, NEC, 128], F16, name="wk_sb")
            wqk2_sb = persist.tile([128, NEC, 128], F16, name="wqk2_sb")
            wv_sb = persist.tile([128, NEC, DSL], F16, name="wv_sb")
            wp_a = persist.tile([128, EMBED], F16, name="wp_a")
            wp_b = persist.tile([128, EMBED], F16, name="wp_b")
            # Q^T/K^T: heads 0,1 at partition halves; head 2 at base 0
            qt01 = persist.tile([128, SEQ], F16, name="qt01")
            kt01 = persist.tile([128, SEQ], F16, name="kt01")
            qt2 = persist.tile([128, SEQ], F16, name="qt2")
            kt2 = persist.tile([128, SEQ], F16, name="kt2")
            # swapped duplicates: [h1 | h0] so every head has Q/K at both
            # partition halves (even k-tiles run at base 0, odd at base 64 ->
            # adjacent matmuls occupy different PE row groups and overlap)
            qtdup = persist.tile([128, SEQ], F16, name="qtdup")
            ktdup = persist.tile([128, SEQ], F16, name="ktdup")
            # V natural layout + ones column: [p, head, kchunk, 66]
            v_sb = persist.tile([128, NHEAD_CORE, NKT, 66], F16, name="v_sb")
            ones_sb = persist.tile([128, 64], F16, name="ones_sb")

            nc.sync.dma_start(out=wq_sb[:], in_=wq_d.rearrange("(c p) d -> p c d", p=128))
            nc.sync.dma_start(out=wk_sb[:], in_=wk_d.rearrange("(c p) d -> p c d", p=128))
            nc.vector.memset(ones_sb[:], 1.0)
            nc.vector.memset(v_sb[:, :, :, 64:66], 1.0)
            # dummy activation: loads the exp table set while ACT is idle
            warm = persist.tile([128, 1], F16, name="warm_sb")
            nc.vector.memset(warm[:], 0.0)
            nc.scalar.activation(out=warm[:], in_=warm[:], func=EXP, scale=1.0)

            # ---- phase 1, pass A: Q^T/K^T for heads 0,1 (gets ACT going fast)
            for sb in range(NQB):  # 8 s-blocks of 512
                qk_ps = psA.tile([128, 1024], F32, name="qkps", tag="psA")
                for e in range(NEC):
                    slab = slabp.tile([128, QB], F16, name="slab", tag="slab")
                    nc.sync.dma_start(
                        out=slab[:],
                        in_=xT_d[128 * e:128 * (e + 1), QB * sb:QB * (sb + 1)],
                    )
                    st = (e == 0)
                    sp = (e == NEC - 1)
                    nc.tensor.matmul(qk_ps[:, 0:QB], wq_sb[:, e, :],
                                     slab[:], start=st, stop=sp)
                    nc.tensor.matmul(qk_ps[:, QB:2 * QB], wk_sb[:, e, :],
                                     slab[:], start=st, stop=sp)
                cols = slice(QB * sb, QB * (sb + 1))
                nc.vector.tensor_copy(qt01[:, cols], qk_ps[:, 0:QB])
                nc.vector.tensor_copy(kt01[:, cols], qk_ps[:, QB:2 * QB])
                nc.vector.tensor_copy(qtdup[64:128, cols], qk_ps[0:64, 0:QB])
                nc.vector.tensor_copy(ktdup[64:128, cols], qk_ps[0:64, QB:2 * QB])

            nc.sync.dma_start(out=wqk2_sb[:], in_=wqk2_d.rearrange("(c p) d -> p c d", p=128))
            nc.sync.dma_start(out=wv_sb[:], in_=wv_d.rearrange("(c p) d -> p c d", p=128))
            nc.sync.dma_start(out=wp_a[:], in_=wp_d[0:128, :])
            nc.sync.dma_start(out=wp_b[0:64, :], in_=wp_d[128:192, :])

            # phase 1, pass B (emitted in bursts inside q-block 0's loop):
            # {Q2|K2} projection + V projection for one s-block
            def pass_b_burst(sb):
                cols = slice(QB * sb, QB * (sb + 1))
                slabs = []
                for e in range(NEC):
                    slab = slabp.tile([128, QB], F16, name="slabB", tag="slab")
                    nc.sync.dma_start(
                        out=slab[:],
                        in_=xT_d[128 * e:128 * (e + 1), cols],
                    )
                    slabs.append(slab)
                qk2_ps = psA.tile([128, 1536], F32, name="qk2ps", tag="psA")
                for e in range(NEC):
                    nc.tensor.matmul(qk2_ps[:, 0:QB], wqk2_sb[:, e, :],
                                     slabs[e][:], start=(e == 0),
                                     stop=(e == NEC - 1))
                nc.vector.tensor_copy(qt2[0:64, cols], qk2_ps[0:64, 0:QB])
                nc.vector.tensor_copy(kt2[0:64, cols], qk2_ps[64:128, 0:QB])
                nc.vector.tensor_copy(qt2[64:128, cols], qk2_ps[0:64, 0:QB])
                nc.vector.tensor_copy(kt2[64:128, cols], qk2_ps[64:128, 0:QB])
                for c in range(4):  # V s-chunks, one PSUM bank at a time
                    v_ps = psC.tile([128, QB], F32, name="vps", tag="psC")
                    for e in range(NEC):
                        nc.tensor.matmul(
                            v_ps[:, 0:DSL],
                            slabs[e][:, 128 * c:128 * (c + 1)],
                            wv_sb[:, e, :],
                            start=(e == 0), stop=(e == NEC - 1))
                    nc.vector.tensor_copy(
                        v_sb[:, :, 4 * sb + c, 0:64],
                        v_ps[:, 0:DSL].rearrange("p (h d) -> p h d", h=NHEAD_CORE),
                    )

            # ---- phase 2: attention + projection ----
            # per-(qb, head) blocks; 3 k-tiles per exp op (3 PSUM banks);
            # attn accumulator spilled to SBUF at block end so psB needs one
            # bank; normalize + projection ride in the next block's slack.
            def normalize_sb(spill, dst):
                """dst = spill[0:64] / spill[64] via recip + K=1 bcast mm."""
                rsb = rsbp.tile([128, QB], F16, name="rsb", tag="rsb")
                with nc.allow_low_precision(reason="fp16 recip feeds bcast mm"):
                    nc.vector.reciprocal(out=rsb[64:65, :], in_=spill[64:65, :])
                ps_bc = psC.tile([128, QB], F32, name="ps_bc", tag="psC")
                nc.tensor.matmul(ps_bc[0:64, :], ones_sb[64:65, 0:64],
                                 rsb[64:65, :], start=True, stop=True)
                bc_sb = bcp.tile([128, QB], F32, name="bc_sb", tag="bc")
                nc.vector.tensor_copy(bc_sb[0:64, :], ps_bc[0:64, :])
                nc.vector.tensor_mul(dst, spill[0:64, :], bc_sb[0:64, :])

            def emit_proj(qb, attA, attB, f):
                qcols = slice(QB * qb, QB * (qb + 1))
                ps_o = psC.tile([128, QB], F32, name="ps_o", tag="psC")
                nc.tensor.matmul(ps_o[:], wp_a[:, 128 * f:128 * (f + 1)],
                                 attA[:], start=True, stop=False)
                nc.tensor.matmul(ps_o[:], wp_b[0:64, 128 * f:128 * (f + 1)],
                                 attB[0:64, :], start=False, stop=True)
                stg = stagep.tile([128, QB], F32, name="stg", tag="stg")
                nc.vector.tensor_copy(stg[:], ps_o[:])
                nc.sync.dma_start(
                    out=yT_d[128 * f:128 * (f + 1), qcols], in_=stg[:])

            # per-head (K even-base-0, Q even, K odd-base-64, Q odd) sources
            def head_srcs(h, kt):
                if kt % 2 == 0:
                    b0 = 0
                    kt_t, qt_t = [(kt01, qt01), (ktdup, qtdup), (kt2, qt2)][h]
                else:
                    b0 = 64
                    kt_t, qt_t = [(ktdup, qtdup), (kt01, qt01), (kt2, qt2)][h]
                return kt_t, qt_t, b0

            GROUPS = [list(range(i, min(i + 3, NKT))) for i in range(0, NKT, 3)]
            att_tiles = {}
            pend_norm = None   # (spill_tile, dst_ap)
            pend_proj = None   # (qb, attA, attB)
            next_burst = 0

            for qb in range(NQB):
                qcols = slice(QB * qb, QB * (qb + 1))
                attA = attp.tile([128, QB], F16, name="attA", tag="attA")
                attB = attp.tile([128, QB], F16, name="attB", tag="attB")
                att_tiles[qb] = (attA, attB)
                for h in range(NHEAD_CORE):
                    ps_att = psB.tile([128, QB], F32, name="ps_att", tag="psB")
                    for gi, group in enumerate(GROUPS):
                        gw = QB * len(group)
                        ps_s = psA.tile([128, 1536], F32, name="ps_s", tag="psA")
                        for i, kt in enumerate(group):
                            kt_t, qt_t, b0 = head_srcs(h, kt)
                            kk = slice(128 * kt, 128 * (kt + 1))
                            nc.tensor.matmul(
                                ps_s[:, QB * i:QB * (i + 1)],
                                kt_t[b0:b0 + 64, kk], qt_t[b0:b0 + 64, qcols],
                                start=True, stop=True)
                        es = esp.tile([128, 1536], F16, name="es", tag="es")
                        nc.scalar.activation(out=es[:, 0:gw], in_=ps_s[:, 0:gw],
                                             func=EXP, scale=SCALE)
                        if qb == 0 and h == 0 and next_burst < NQB:
                            # burst sb covers V k-chunks up to 4*sb+3 >=
                            # group[-1]=3*sb+2, always ahead of the attnV
                            pass_b_burst(next_burst)
                            next_burst += 1
                        for i, kt in enumerate(group):
                            nc.tensor.matmul(
                                ps_att[0:65, :], v_sb[:, h, kt, 0:65],
                                es[:, QB * i:QB * (i + 1)],
                                start=(kt == 0), stop=(kt == NKT - 1),
                                skip_group_check=True)
                        if qb == 0 and h == 0 and 2 <= gi < 6:
                            # h1-side Q/K duplicates, first needed next block
                            srcs = [(qtdup, qt01), (ktdup, kt01)]
                            dt_, st_ = srcs[(gi - 2) % 2]
                            half = slice(0, SEQ // 2) if gi < 4 else slice(SEQ // 2, SEQ)
                            nc.vector.tensor_copy(dt_[0:64, half],
                                                  st_[64:128, half])
                        if gi == 1 and pend_norm is not None:
                            normalize_sb(*pend_norm)
                            pend_norm = None
                        if pend_proj is not None and 3 <= gi < 3 + NFT:
                            emit_proj(pend_proj[0], pend_proj[1], pend_proj[2],
                                      gi - 3)
                            if gi == 3 + NFT - 1:
                                pend_proj = None
                    # spill accumulator to SBUF; frees the psB bank quickly
                    spill = spillp.tile([128, QB], F32, name="spill", tag="spill")
                    nc.vector.tensor_copy(spill[0:65, :], ps_att[0:65, :])
                    if h == 0:
                        dst = attA[0:64, :]
                    elif h == 1:
                        dst = attA[64:128, :]
                    else:
                        dst = attB[0:64, :]
                    pend_norm = (spill, dst)
                    if h == 2:
                        pend_proj = (qb, attA, attB)

            normalize_sb(*pend_norm)
            for f in range(NFT):
                emit_proj(pend_proj[0], pend_proj[1], pend_proj[2], f)

    nc.compile()
    return nc


def _get_nc():
    if "nc" not in _CACHED:
        _CACHED["nc"] = _build()
    return _CACHED["nc"]


def _make_in_maps(x, W_qkv, W_proj):
    f16 = np.float16
    in_maps = []
    for c in range(8):
        b = c // 4
        g = c % 4
        sl = slice(DSL * g, DSL * (g + 1))
        xT = np.ascontiguousarray(x[b].T).astype(f16)
        wqT = np.ascontiguousarray(W_qkv[0:EMBED][sl, :].T)         # [768,192]
        wkT = np.ascontiguousarray(W_qkv[EMBED:2 * EMBED][sl, :].T)
        wvT = np.ascontiguousarray(W_qkv[2 * EMBED:3 * EMBED][sl, :].T)
        wp = np.ascontiguousarray(W_proj[:, sl].T)                  # [192,768]
        wqk2 = np.concatenate([wqT[:, 128:192], wkT[:, 128:192]], axis=1)
        in_maps.append({
            "xT": xT,
            "wq": wqT[:, 0:128].astype(f16),
            "wk": wkT[:, 0:128].astype(f16),
            "wqk2": np.ascontiguousarray(wqk2).astype(f16),
            "wv": wvT.astype(f16),
            "wp": wp.astype(f16),
        })
    return in_maps


def kernel(x, W_qkv, W_proj, b_proj):
    from concourse.bass_utils import run_bass_kernel_spmd

    x = np.asarray(x, dtype=np.float32)
    W_qkv = np.asarray(W_qkv, dtype=np.float32)
    W_proj = np.asarray(W_proj, dtype=np.float32)
    b_proj = np.asarray(b_proj, dtype=np.float32)

    nc = _get_nc()
    in_maps = _make_in_maps(x, W_qkv, W_proj)
    res = run_bass_kernel_spmd(nc, in_maps, core_ids=list(range(8)))

    y = np.zeros((2, SEQ, EMBED), dtype=np.float32)
    for c in range(8):
        y[c // 4] += res.results[c]["yT"].T
    y += b_proj
    return y
